# revision 7
# baseline (speedup 1.0000x reference)
"""Bass/Tile kernel for nn_MultiHeadAttention (B=2, S=2048, D=1024, H=16) on 8 trn2 cores.

Sharding: core c -> (b = c//4, head-group hg = c%4). Each core computes 4 heads'
q/k/v projections, relu-attention, and a partial FC (256 of 1024 contraction rows).
Host pre-casts to bf16, pre-transposes x / weight slices, and sums the 4
partials per batch + bias.

v3 design (all-bf16 compute, fp32 PSUM accumulate):
  - scores: K=64 row-tiled head pairs writing one 2-bank PSUM tile;
    a single 1024-wide relu drains the pair (halves elementwise op count)
  - attn@v: M=64 col-tiled head pairs accumulating into bank halves
  - emission interleaves projection / FC matmul groups into the attention
    m-loops so the PE queue is dense (keeps HAM warm) while DVE+ACT chew relus
  - schedule: k(c0) -> [scores(qb0,h0) + v]-interleave -> [av(qb0,h0) + k(c1)]
    -> scores/av(qb0,h1) -> qb 1..3 with fc(qb-1) injected -> fc(qb3) tail
"""
import numpy as np
import ml_dtypes

import concourse.bass as bass
import concourse.mybir as mybir
import concourse.tile as tile

F32 = mybir.dt.float32
BF16 = mybir.dt.bfloat16
ts, ds = bass.ts, bass.ds

S = 2048
D = 1024
DL = 256      # per-core q/k/v dim (4 heads x 64)
P = 128
KD = D // P   # 8 k-chunks for projections
SQ = 512      # q-block (matmul N)
NQB = S // SQ # 4
NM = S // P   # 16 kpos chunks
DLC = DL // P # 2


def split_excess_waits(nc, max_embed: int = 1):
    """walrus core_v3 codegen accepts at most one sync-wait per instruction;
    move extra waits onto standalone event-sem instructions inserted before."""
    n_split = 0
    counter = 0
    for f in nc.m.functions:
        for blk in f.blocks:
            insts = blk.instructions
            if not any(
                ins.sync_info is not None and len(ins.sync_info.on_wait) > max_embed
                for ins in insts
            ):
                continue
            newl = []
            for ins in insts:
                si = ins.sync_info
                if si is not None and len(si.on_wait) > max_embed:
                    waits = list(si.on_wait)
                    extra, keep = waits[:-max_embed], waits[-max_embed:]
                    for w in extra:
                        counter += 1
                        es = mybir.InstEventSemaphore(name=f"waitsplit_{counter}")
                        es.engine = ins.engine
                        es.sync_info = mybir.SyncInfo(on_wait=[w], on_update=[])
                        newl.append(es)
                        n_split += 1
                    si.on_wait = keep
                newl.append(ins)
            blk.instructions = newl
    return n_split


def build_nc(with_mask: bool):
    nc = bass.Bass()
    # pre-arranged on host: x[p, c, s] = x.T[128c+p, s]; w[p, c, f] = w.T[128c+p, f]
    xT = nc.dram_tensor("xT", [P, KD, S], BF16, kind="ExternalInput")
    wq = nc.dram_tensor("wq", [P, KD, DL], BF16, kind="ExternalInput")
    wk = nc.dram_tensor("wk", [P, KD, DL], BF16, kind="ExternalInput")
    wv = nc.dram_tensor("wv", [P, KD, DL], BF16, kind="ExternalInput")
    wfc = nc.dram_tensor("wfc", [P, DLC, D], BF16, kind="ExternalInput")
    maskT = nc.dram_tensor("maskT", [S, S], F32, kind="ExternalInput") if with_mask else None
    # bf16 output halves the store traffic; partials are summed in fp64 on host
    y = nc.dram_tensor("y", [S, D], BF16, kind="ExternalOutput")

    with tile.TileContext(nc) as tc:
        _Emitter(tc, xT, wq, wk, wv, wfc, maskT, y).run()
    split_excess_waits(nc)
    return nc


class _Emitter:
    def __init__(self, tc, xT, wq, wk, wv, wfc, maskT, y):
        self.tc = tc
        self.nc = tc.nc
        self.xT, self.wq, self.wk, self.wv, self.wfc = xT, wq, wk, wv, wfc
        self.maskT, self.y = maskT, y
        self.cb = 0
        self.cp = 0
        self.rl = 0
        self.dq = 0

    # -- engine alternation helpers ----------------------------------------
    def dma(self, out_ap, in_ap):
        eng = (self.nc.sync, self.nc.gpsimd, self.nc.scalar)[self.dq % 3]
        eng.dma_start(out_ap, in_ap)
        self.dq += 1

    def copyback(self, out_ap, in_ap):
        if self.cp % 2 == 0:
            self.nc.vector.tensor_copy(out_ap, in_ap)
        else:
            self.nc.scalar.copy(out_ap, in_ap)
        self.cp += 1

    def relu(self, out_ap, in_ap):
        if self.rl % 2 == 0:
            self.nc.vector.tensor_scalar_max(out_ap, in_ap, 0.0)
        else:
            self.nc.scalar.activation(out_ap, in_ap, mybir.ActivationFunctionType.Relu)
        self.rl += 1

    # -- emission pieces ----------------------------------------------------
    def kq_group(self, wsb, dstT, c, nb, pool=None):
        """one projection psum group: dstT[:, c, nb*SQ:...] via 8 k-chunk matmuls"""
        nc = self.nc
        pool = pool if pool is not None else self.ps
        pt = pool.tile([P, SQ], F32, tag="pp" if pool is self.ps else "fc",
                       name=f"pj_{dstT.name}_{c}_{nb}")
        for k in range(KD):
            nc.tensor.matmul(
                pt[:], wsb[:, k, ts(c, P)], self.xb[:, k, ds(nb * SQ, SQ)],
                start=(k == 0), stop=(k == KD - 1),
            )
        self.copyback(dstT[:, c, ds(nb * SQ, SQ)], pt[:])

    def v_group(self, sc):
        nc = self.nc
        pt = self.ps.tile([P, DL], F32, tag="pp", name=f"v_{sc}")
        for k in range(KD):
            nc.tensor.matmul(
                pt[:], self.xb[:, k, ts(sc, P)], self.wv_sb[:, k, :],
                start=(k == 0), stop=(k == KD - 1),
            )
        self.copyback(self.vN[:, sc, :], pt[:])

    def scores(self, qb, hp, m, attn_t, mtile):
        nc = self.nc
        pt2 = self.ps_sc.tile([P, 2, SQ], F32, tag="sc", name=f"sc_{qb}_{hp}_{m}")
        for h in range(2):
            nc.tensor.matmul(
                pt2[:, h, :],
                self.kT[ds(64 * h, 64), hp, ts(m, P)],
                self.qT[ds(64 * h, 64), hp, ds(qb * SQ, SQ)],
                start=True, stop=True,
            )
        if mtile is not None:
            nc.vector.tensor_tensor(
                pt2[:, 0, :], pt2[:, 0, :], mtile[:, m, :], mybir.AluOpType.add
            )
            nc.vector.tensor_tensor(
                pt2[:, 1, :], pt2[:, 1, :], mtile[:, m, :], mybir.AluOpType.add
            )
        self.relu(attn_t[:, m, :, :], pt2[:, :, :])

    def scores_single(self, qb, hp, m, attn_t, mtile):
        nc = self.nc
        pts = []
        for h in range(2):
            pt = self.ps.tile([P, SQ], F32, tag="pp", name=f"sc_{qb}_{hp}_{m}_{h}")
            nc.tensor.matmul(
                pt[:],
                self.kT[ds(64 * h, 64), hp, ts(m, P)],
                self.qT[ds(64 * h, 64), hp, ds(qb * SQ, SQ)],
                start=True, stop=True,
            )
            pts.append(pt)
        for h in range(2):
            pt = pts[h]
            if mtile is not None:
                nc.vector.tensor_tensor(
                    pt[:], pt[:], mtile[:, m, :], mybir.AluOpType.add
                )
            self.relu(attn_t[:, m, h, :], pt[:])

    def av(self, qb, hp, m, attn_t, po):
        nc = self.nc
        for h in range(2):
            nc.tensor.matmul(
                po[ds(64 * h, 64), :],
                self.vN[:, m, ds(128 * hp + 64 * h, 64)],
                attn_t[:, m, h, :],
                start=(m == 0), stop=(m == NM - 1),
            )

    def fc_group(self, sc, eb, pool=None):
        nc = self.nc
        pool = pool if pool is not None else self.ps_fc
        pt = pool.tile([P, SQ], F32, tag="fc" if pool is self.ps_fc else "pp",
                       name=f"fc_{sc}_{eb}")
        for c in range(DLC):
            nc.tensor.matmul(
                pt[:], self.outT[:, c, ts(sc, P)], self.wfc_sb[:, c, ds(eb * SQ, SQ)],
                start=(c == 0), stop=(c == DLC - 1),
            )
        yt = self.ystage.tile([P, SQ], BF16, tag="yt", name=f"yt_{sc}_{eb}")
        self.copyback(yt[:], pt[:])
        nc.sync.dma_start(self.y[ts(sc, P), ds(eb * SQ, SQ)], yt[:])

    def inject(self, n=1):
        """pop pending PE work (fc groups / q-projections) into the stream"""
        for _ in range(n):
            if self.pe_pending:
                self.pe_pending.pop(0)()

    def load_mask(self, qb):
        if self.maskT is None:
            return None
        nc = self.nc
        mtile = self.mstg.tile([P, NM, SQ], F32, tag="mask", name=f"mask_{qb}")
        for m in range(NM):
            nc.sync.dma_start(
                mtile[:, m, :],
                self.maskT[:, :].rearrange("(m p) q -> p m q", p=P)[:, m, ds(qb * SQ, SQ)],
            )
        return mtile

    def attn_tile(self, qb, hp):
        return self.attn_pool.tile(
            [P, NM, 2, SQ], BF16, tag="attn", name=f"attn_{qb}_{hp}"
        )

    def av_tile(self, qb, hp):
        return self.ps_av.tile([P, SQ], F32, tag="av", name=f"av_{qb}_{hp}")

    # -- main ---------------------------------------------------------------
    def run(self):
        from contextlib import ExitStack

        tc, nc = self.tc, self.nc
        stack = ExitStack()
        sb = stack.enter_context(tc.tile_pool(name="sb", bufs=1))
        # PSUM budget (8 banks): pp 4, fc 2, av 2
        self.ps = stack.enter_context(tc.tile_pool(name="ps", bufs=4, space="PSUM"))
        self.ps_fc = stack.enter_context(tc.tile_pool(name="ps_fc", bufs=2, space="PSUM"))
        self.ps_av = stack.enter_context(tc.tile_pool(name="ps_av", bufs=2, space="PSUM"))
        self.attn_pool = stack.enter_context(tc.tile_pool(name="attn", bufs=2))
        self.mstg = stack.enter_context(tc.tile_pool(name="mstg", bufs=2))
        self.ystage = stack.enter_context(tc.tile_pool(name="ystage", bufs=4))

        self.xb = sb.tile([P, KD, S], BF16, name="xb")
        self.wq_sb = sb.tile([P, KD, DL], BF16, name="wq_sb")
        self.wk_sb = sb.tile([P, KD, DL], BF16, name="wk_sb")
        self.wv_sb = sb.tile([P, KD, DL], BF16, name="wv_sb")
        self.wfc_sb = sb.tile([P, DLC, D], BF16, name="wfc_sb")
        self.qT = sb.tile([P, DLC, S], BF16, name="qT")
        self.kT = sb.tile([P, DLC, S], BF16, name="kT")
        self.vN = sb.tile([P, NM, DL], BF16, name="vN")
        self.outT = sb.tile([P, DLC, S], BF16, name="outT")
        self.pe_pending = []

        # loads in consumption order: wk chunks, then x nb-block-major in
        # 128KB (k, nb) chunks so the first k-projection starts ~0.5us in,
        # then wv / wq / wfc (needed 14us+ / 28us+ / 50us+ into the run)
        for k in range(KD):
            self.dma(self.wk_sb[:, k, :], self.wk[:, k, :])
        for nb in range(NQB):
            for k in range(KD):
                self.dma(
                    self.xb[:, k, ds(nb * SQ, SQ)],
                    self.xT[:, k, ds(nb * SQ, SQ)],
                )
        for k in range(KD):
            self.dma(self.wv_sb[:, k, :], self.wv[:, k, :])
        for k in range(KD):
            self.dma(self.wq_sb[:, k, :], self.wq[:, k, :])
        for c in range(DLC):
            self.dma(self.wfc_sb[:, c, :], self.wfc[:, c, :])

        # k-projection nb-outer to match the x stream, then v-projection
        for nb in range(NQB):
            for c in range(DLC):
                self.kq_group(self.wk_sb, self.kT, c, nb)
        for sc in range(NM):
            self.v_group(sc)

        # per q-block: attention per head pair; fc of qb-1 and q-projection
        # of qb+1 are injected into the m-loops so the PE stream stays dense
        for c in range(DLC):
            self.kq_group(self.wq_sb, self.qT, c, 0)
        for qb in range(NQB):
            mt = self.load_mask(qb)
            for hp in range(DLC):
                at = self.attn_tile(qb, hp)
                po = self.av_tile(qb, hp)
                for m in range(NM + 2):
                    if m < NM:
                        self.scores_single(qb, hp, m, at, mt)
                    if m >= 2:
                        self.av(qb, hp, m - 2, at, po)
                    if m % 2 == 1:
                        self.inject(1)
                last = qb == NQB - 1 and hp == DLC - 1
                if last:
                    # final copyback is on the critical path: split across engines
                    nc.vector.tensor_copy(
                        self.outT[:, hp, ds(qb * SQ, SQ // 2)], po[:, 0:SQ // 2])
                    nc.scalar.copy(
                        self.outT[:, hp, ds(qb * SQ + SQ // 2, SQ // 2)],
                        po[:, SQ // 2:SQ])
                else:
                    self.copyback(self.outT[:, hp, ds(qb * SQ, SQ)], po[:])
                if hp == 0 and qb < NQB - 1:
                    qn = qb + 1
                    self.pe_pending[:0] = [
                        (lambda c=c, qn=qn: self.kq_group(
                            self.wq_sb, self.qT, c, qn, pool=self.ps_fc))
                        for c in range(DLC)
                    ]
            tail = qb == NQB - 1
            self.pe_pending += [
                (lambda sc=sc, eb=eb, i=i, tail=tail: self.fc_group(
                    sc, eb, pool=(self.ps if (tail and i % 2 == 0) else self.ps_fc)))
                for i, (sc, eb) in enumerate(
                    (sc, eb) for sc in range(qb * 4, qb * 4 + 4) for eb in range(2))
            ]
        while self.pe_pending:
            self.inject(1)

        stack.close()


# ---- host wrapper ---------------------------------------------------------

N_HEAD = 16
_nc_cache = {}


def get_nc(with_mask: bool):
    if with_mask not in _nc_cache:
        _nc_cache[with_mask] = build_nc(with_mask)
    return _nc_cache[with_mask]


def make_in_maps(x, mask, Wq, Wk, Wv, Wfc, with_mask):
    scale = np.float32(1.0 / np.sqrt(D // N_HEAD))
    bf = ml_dtypes.bfloat16
    in_maps = []
    for c in range(8):
        b, hg = divmod(c, 4)
        gs = slice(DL * hg, DL * hg + DL)
        def prearrange(wT, cdim):  # [cdim*128, F] -> [128, cdim, F]
            F = wT.shape[1]
            return np.ascontiguousarray(
                wT.reshape(cdim, P, F).transpose(1, 0, 2)
            ).astype(bf)

        m = {
            "xT": prearrange(x[b].T, KD),
            "wq": prearrange((Wq[gs, :] * scale).T, KD),
            "wk": prearrange(Wk[gs, :].T, KD),
            "wv": prearrange(Wv[gs, :].T, KD),
            "wfc": prearrange(Wfc[:, gs].T, DLC),
        }
        if with_mask:
            m["maskT"] = np.ascontiguousarray(
                np.broadcast_to(mask, (1, 1, S, S))[0, 0].T.astype(np.float32)
            )
        in_maps.append(m)
    return in_maps


def kernel(x, mask, Wq, Wk, Wv, Wfc, bfc):
    """Full-input entry: shards across 8 trn2 cores, returns the full output."""
    from concourse.bass_utils import run_bass_kernel_spmd

    x = np.asarray(x, dtype=np.float32)
    mask = np.asarray(mask, dtype=np.float32)
    Wq = np.asarray(Wq, dtype=np.float32)
    Wk = np.asarray(Wk, dtype=np.float32)
    Wv = np.asarray(Wv, dtype=np.float32)
    Wfc = np.asarray(Wfc, dtype=np.float32)
    bfc = np.asarray(bfc, dtype=np.float32)

    B = x.shape[0]
    with_mask = bool(np.any(mask))
    nc = get_nc(with_mask)
    in_maps = make_in_maps(x, mask, Wq, Wk, Wv, Wfc, with_mask)

    res = run_bass_kernel_spmd(nc, in_maps, core_ids=list(range(8)))
    parts = np.stack([np.asarray(r["y"], dtype=np.float64) for r in res.results])
    out = parts.reshape(B, 4, S, D).sum(axis=1)
    out += bfc.astype(np.float64)
    return out.astype(np.float32)



# revision 12
# speedup vs baseline: 1.2167x; 1.2167x over previous
"""Bass/Tile kernel for nn_MultiHeadAttention (B=2, S=2048, D=1024, H=16) on 8 trn2 cores.

Sharding: core c -> (b = c//4, head-group hg = c%4). Each core computes 4 heads'
q/k/v projections, relu-attention, and a partial FC (256 of 1024 contraction rows).
Host pre-casts to bf16, pre-transposes x / weight slices, and sums the 4
partials per batch + bias.

v3 design (all-bf16 compute, fp32 PSUM accumulate):
  - scores: K=64 row-tiled head pairs writing one 2-bank PSUM tile;
    a single 1024-wide relu drains the pair (halves elementwise op count)
  - attn@v: M=64 col-tiled head pairs accumulating into bank halves
  - emission interleaves projection / FC matmul groups into the attention
    m-loops so the PE queue is dense (keeps HAM warm) while DVE+ACT chew relus
  - schedule: k(c0) -> [scores(qb0,h0) + v]-interleave -> [av(qb0,h0) + k(c1)]
    -> scores/av(qb0,h1) -> qb 1..3 with fc(qb-1) injected -> fc(qb3) tail
"""
import numpy as np
import ml_dtypes

import concourse.bass as bass
import concourse.mybir as mybir
import concourse.tile as tile

F32 = mybir.dt.float32
BF16 = mybir.dt.bfloat16
ts, ds = bass.ts, bass.ds

S = 2048
D = 1024
DL = 256      # per-core q/k/v dim (4 heads x 64)
P = 128
KD = D // P   # 8 k-chunks for projections
SQ = 512      # q-block (matmul N)
NQB = S // SQ # 4
NM = S // P   # 16 kpos chunks
DLC = DL // P # 2


def split_excess_waits(nc, max_embed: int = 1):
    """walrus core_v3 codegen accepts at most one sync-wait per instruction;
    move extra waits onto standalone event-sem instructions inserted before."""
    n_split = 0
    counter = 0
    for f in nc.m.functions:
        for blk in f.blocks:
            insts = blk.instructions
            if not any(
                ins.sync_info is not None and len(ins.sync_info.on_wait) > max_embed
                for ins in insts
            ):
                continue
            newl = []
            for ins in insts:
                si = ins.sync_info
                if si is not None and len(si.on_wait) > max_embed:
                    waits = list(si.on_wait)
                    extra, keep = waits[:-max_embed], waits[-max_embed:]
                    for w in extra:
                        counter += 1
                        es = mybir.InstEventSemaphore(name=f"waitsplit_{counter}")
                        es.engine = ins.engine
                        es.sync_info = mybir.SyncInfo(on_wait=[w], on_update=[])
                        newl.append(es)
                        n_split += 1
                    si.on_wait = keep
                newl.append(ins)
            blk.instructions = newl
    return n_split


def build_nc(with_mask: bool):
    nc = bass.Bass()
    # pre-arranged on host: x[p, nb, k, sq] = x.T[128k+p, 512nb+sq] (nb-block
    # granular so one 512KB DMA with 4KB lines unlocks one q-block of work);
    # w[p, c, f] = w.T[128c+p, f]
    xT = nc.dram_tensor("xT", [P, NQB, KD, SQ], BF16, kind="ExternalInput")
    wq = nc.dram_tensor("wq", [P, KD, DL], BF16, kind="ExternalInput")
    wk = nc.dram_tensor("wk", [P, KD, DL], BF16, kind="ExternalInput")
    wv = nc.dram_tensor("wv", [P, KD, DL], BF16, kind="ExternalInput")
    wfc = nc.dram_tensor("wfc", [P, DLC, D], BF16, kind="ExternalInput")
    maskT = nc.dram_tensor("maskT", [S, S], F32, kind="ExternalInput") if with_mask else None
    # bf16 output halves the store traffic; partials are summed in fp64 on host
    y = nc.dram_tensor("y", [S, D], BF16, kind="ExternalOutput")

    with tile.TileContext(nc) as tc:
        _Emitter(tc, xT, wq, wk, wv, wfc, maskT, y).run()
    split_excess_waits(nc)
    return nc


class _Emitter:
    def __init__(self, tc, xT, wq, wk, wv, wfc, maskT, y):
        self.tc = tc
        self.nc = tc.nc
        self.xT, self.wq, self.wk, self.wv, self.wfc = xT, wq, wk, wv, wfc
        self.maskT, self.y = maskT, y
        self.cb = 0
        self.cp = 0
        self.rl = 0
        self.dq = 0

    # -- engine alternation helpers ----------------------------------------
    def dma(self, out_ap, in_ap):
        eng = (self.nc.sync, self.nc.gpsimd, self.nc.scalar)[self.dq % 3]
        eng.dma_start(out_ap, in_ap)
        self.dq += 1

    def copyback(self, out_ap, in_ap):
        if self.cp % 2 == 0:
            self.nc.vector.tensor_copy(out_ap, in_ap)
        else:
            self.nc.scalar.copy(out_ap, in_ap)
        self.cp += 1

    def relu(self, out_ap, in_ap):
        if self.rl % 2 == 0:
            self.nc.vector.tensor_scalar_max(out_ap, in_ap, 0.0)
        else:
            self.nc.scalar.activation(out_ap, in_ap, mybir.ActivationFunctionType.Relu)
        self.rl += 1

    # -- emission pieces ----------------------------------------------------
    def kq_group(self, wsb, dstT, c, nb, pool=None):
        """one projection psum group: dstT[:, c, nb*SQ:...] via 8 k-chunk matmuls"""
        nc = self.nc
        pool = pool if pool is not None else self.ps
        pt = pool.tile([P, SQ], F32, tag="pp" if pool is self.ps else "fc",
                       name=f"pj_{dstT.name}_{c}_{nb}")
        for k in range(KD):
            nc.tensor.matmul(
                pt[:], wsb[:, k, ts(c, P)], self.xb[:, nb, k, :],
                start=(k == 0), stop=(k == KD - 1),
            )
        self.copyback(dstT[:, c, ds(nb * SQ, SQ)], pt[:])

    def v_group(self, sc):
        nc = self.nc
        pt = self.ps.tile([P, DL], F32, tag="pp", name=f"v_{sc}")
        for k in range(KD):
            nc.tensor.matmul(
                pt[:], self.xb[:, sc // 4, k, ds((sc % 4) * P, P)], self.wv_sb[:, k, :],
                start=(k == 0), stop=(k == KD - 1),
            )
        self.copyback(self.vN[:, sc, :], pt[:])

    def scores(self, qb, hp, m, attn_t, mtile):
        nc = self.nc
        pt2 = self.ps_sc.tile([P, 2, SQ], F32, tag="sc", name=f"sc_{qb}_{hp}_{m}")
        for h in range(2):
            nc.tensor.matmul(
                pt2[:, h, :],
                self.kT[ds(64 * h, 64), hp, ts(m, P)],
                self.qT[ds(64 * h, 64), hp, ds(qb * SQ, SQ)],
                start=True, stop=True,
            )
        if mtile is not None:
            nc.vector.tensor_tensor(
                pt2[:, 0, :], pt2[:, 0, :], mtile[:, m, :], mybir.AluOpType.add
            )
            nc.vector.tensor_tensor(
                pt2[:, 1, :], pt2[:, 1, :], mtile[:, m, :], mybir.AluOpType.add
            )
        self.relu(attn_t[:, m, :, :], pt2[:, :, :])

    def scores_single(self, qb, hp, m, attn_t, mtile):
        nc = self.nc
        pts = []
        for h in range(2):
            pt = self.ps.tile([P, SQ], F32, tag="pp", name=f"sc_{qb}_{hp}_{m}_{h}")
            nc.tensor.matmul(
                pt[:],
                self.kT[ds(64 * h, 64), hp, ts(m, P)],
                self.qT[ds(64 * h, 64), hp, ds(qb * SQ, SQ)],
                start=True, stop=True,
            )
            pts.append(pt)
        for h in range(2):
            pt = pts[h]
            if mtile is not None:
                nc.vector.tensor_tensor(
                    pt[:], pt[:], mtile[:, m, :], mybir.AluOpType.add
                )
            self.relu(attn_t[:, m, h, :], pt[:])

    def av(self, qb, hp, m, attn_t, po):
        nc = self.nc
        for h in range(2):
            nc.tensor.matmul(
                po[ds(64 * h, 64), :],
                self.vN[:, m, ds(128 * hp + 64 * h, 64)],
                attn_t[:, m, h, :],
                start=(m == 0), stop=(m == NM - 1),
            )

    def fc_group(self, sc, eb, pool=None):
        nc = self.nc
        pool = pool if pool is not None else self.ps_fc
        pt = pool.tile([P, SQ], F32, tag="fc" if pool is self.ps_fc else "pp",
                       name=f"fc_{sc}_{eb}")
        for c in range(DLC):
            nc.tensor.matmul(
                pt[:], self.outT[:, c, ts(sc, P)], self.wfc_sb[:, c, ds(eb * SQ, SQ)],
                start=(c == 0), stop=(c == DLC - 1),
            )
        yt = self.ystage.tile([P, SQ], BF16, tag="yt", name=f"yt_{sc}_{eb}")
        self.copyback(yt[:], pt[:])
        nc.sync.dma_start(self.y[ts(sc, P), ds(eb * SQ, SQ)], yt[:])

    def inject(self, n=1):
        """pop pending PE work (fc groups / q-projections) into the stream"""
        for _ in range(n):
            if self.pe_pending:
                self.pe_pending.pop(0)()

    def load_mask(self, qb):
        if self.maskT is None:
            return None
        nc = self.nc
        mtile = self.mstg.tile([P, NM, SQ], F32, tag="mask", name=f"mask_{qb}")
        for m in range(NM):
            nc.sync.dma_start(
                mtile[:, m, :],
                self.maskT[:, :].rearrange("(m p) q -> p m q", p=P)[:, m, ds(qb * SQ, SQ)],
            )
        return mtile

    def attn_tile(self, qb, hp):
        return self.attn_pool.tile(
            [P, NM, 2, SQ], BF16, tag="attn", name=f"attn_{qb}_{hp}"
        )

    def av_tile(self, qb, hp):
        return self.ps_av.tile([P, SQ], F32, tag="av", name=f"av_{qb}_{hp}")

    # -- main ---------------------------------------------------------------
    def run(self):
        from contextlib import ExitStack

        tc, nc = self.tc, self.nc
        stack = ExitStack()
        sb = stack.enter_context(tc.tile_pool(name="sb", bufs=1))
        # PSUM budget (8 banks): pp 4, fc 2, av 2
        self.ps = stack.enter_context(tc.tile_pool(name="ps", bufs=4, space="PSUM"))
        self.ps_fc = stack.enter_context(tc.tile_pool(name="ps_fc", bufs=2, space="PSUM"))
        self.ps_av = stack.enter_context(tc.tile_pool(name="ps_av", bufs=2, space="PSUM"))
        self.attn_pool = stack.enter_context(tc.tile_pool(name="attn", bufs=2))
        self.mstg = stack.enter_context(tc.tile_pool(name="mstg", bufs=2))
        self.ystage = stack.enter_context(tc.tile_pool(name="ystage", bufs=4))

        self.xb = sb.tile([P, NQB, KD, SQ], BF16, name="xb")
        self.wq_sb = sb.tile([P, KD, DL], BF16, name="wq_sb")
        self.wk_sb = sb.tile([P, KD, DL], BF16, name="wk_sb")
        self.wv_sb = sb.tile([P, KD, DL], BF16, name="wv_sb")
        self.wfc_sb = sb.tile([P, DLC, D], BF16, name="wfc_sb")
        self.qT = sb.tile([P, DLC, S], BF16, name="qT")
        self.kT = sb.tile([P, DLC, S], BF16, name="kT")
        self.vN = sb.tile([P, NM, DL], BF16, name="vN")
        self.outT = sb.tile([P, DLC, S], BF16, name="outT")
        self.pe_pending = []

        # loads: wk + wv up front on gpsimd's queue; x streams nb-block-major
        # as 512KB half-k transfers (4KB lines) on sync/scalar queues.
        # wq / wfc are only needed ~29us / ~60us in: their DMAs are gated on
        # a tiny memset (WAW hazard) placed later in the vector stream so the
        # startup HBM bandwidth goes to x.
        nc.gpsimd.dma_start(self.wk_sb[:], self.wk[:, :, :])
        nc.gpsimd.dma_start(self.wv_sb[:], self.wv[:, :, :])
        for nb in range(NQB):
            for kh in range(2):
                eng = (nc.sync, nc.scalar)[(nb * 2 + kh) % 2]
                eng.dma_start(
                    self.xb[:, nb, ds(kh * 4, 4), :],
                    self.xT[:, nb, ds(kh * 4, 4), :],
                )

        # k-projection nb-outer to match the x stream, then v-projection
        for nb in range(NQB):
            for c in range(DLC):
                self.kq_group(self.wk_sb, self.kT, c, nb)
            if nb == 1:
                nc.vector.memset(self.wq_sb[0:1, 0, 0:1], 0.0)
                nc.gpsimd.dma_start(self.wq_sb[:], self.wq[:, :, :])
        for sc in range(NM):
            self.v_group(sc)
            if sc == 3:
                nc.vector.memset(self.wfc_sb[0:1, 0, 0:1], 0.0)
                nc.gpsimd.dma_start(self.wfc_sb[:], self.wfc[:, :, :])

        # per q-block: attention per head pair; fc of qb-1 and q-projection
        # of qb+1 are injected into the m-loops so the PE stream stays dense
        for c in range(DLC):
            self.kq_group(self.wq_sb, self.qT, c, 0)
        for qb in range(NQB):
            mt = self.load_mask(qb)
            for hp in range(DLC):
                at = self.attn_tile(qb, hp)
                po = self.av_tile(qb, hp)
                for m in range(NM + 2):
                    if m < NM:
                        self.scores_single(qb, hp, m, at, mt)
                    if m >= 2:
                        self.av(qb, hp, m - 2, at, po)
                    if m % 2 == 1:
                        self.inject(1)
                last = qb == NQB - 1 and hp == DLC - 1
                if last:
                    # final copyback is on the critical path: split across engines
                    nc.vector.tensor_copy(
                        self.outT[:, hp, ds(qb * SQ, SQ // 2)], po[:, 0:SQ // 2])
                    nc.scalar.copy(
                        self.outT[:, hp, ds(qb * SQ + SQ // 2, SQ // 2)],
                        po[:, SQ // 2:SQ])
                else:
                    self.copyback(self.outT[:, hp, ds(qb * SQ, SQ)], po[:])
                if hp == 0 and qb < NQB - 1:
                    qn = qb + 1
                    self.pe_pending[:0] = [
                        (lambda c=c, qn=qn: self.kq_group(
                            self.wq_sb, self.qT, c, qn, pool=self.ps_fc))
                        for c in range(DLC)
                    ]
            tail = qb == NQB - 1
            self.pe_pending += [
                (lambda sc=sc, eb=eb, i=i, tail=tail: self.fc_group(
                    sc, eb, pool=(self.ps if (tail and i % 2 == 0) else self.ps_fc)))
                for i, (sc, eb) in enumerate(
                    (sc, eb) for sc in range(qb * 4, qb * 4 + 4) for eb in range(2))
            ]
        while self.pe_pending:
            self.inject(1)

        stack.close()


# ---- host wrapper ---------------------------------------------------------

N_HEAD = 16
_nc_cache = {}


def get_nc(with_mask: bool):
    if with_mask not in _nc_cache:
        _nc_cache[with_mask] = build_nc(with_mask)
    return _nc_cache[with_mask]


def make_in_maps(x, mask, Wq, Wk, Wv, Wfc, with_mask):
    scale = np.float32(1.0 / np.sqrt(D // N_HEAD))
    bf = ml_dtypes.bfloat16
    in_maps = []
    for c in range(8):
        b, hg = divmod(c, 4)
        gs = slice(DL * hg, DL * hg + DL)
        def prearrange(wT, cdim):  # [cdim*128, F] -> [128, cdim, F]
            F = wT.shape[1]
            return np.ascontiguousarray(
                wT.reshape(cdim, P, F).transpose(1, 0, 2)
            ).astype(bf)

        xt = x[b].T.reshape(KD, P, NQB, SQ).transpose(1, 2, 0, 3)
        m = {
            "xT": np.ascontiguousarray(xt).astype(bf),
            "wq": prearrange((Wq[gs, :] * scale).T, KD),
            "wk": prearrange(Wk[gs, :].T, KD),
            "wv": prearrange(Wv[gs, :].T, KD),
            "wfc": prearrange(Wfc[:, gs].T, DLC),
        }
        if with_mask:
            m["maskT"] = np.ascontiguousarray(
                np.broadcast_to(mask, (1, 1, S, S))[0, 0].T.astype(np.float32)
            )
        in_maps.append(m)
    return in_maps


def kernel(x, mask, Wq, Wk, Wv, Wfc, bfc):
    """Full-input entry: shards across 8 trn2 cores, returns the full output."""
    from concourse.bass_utils import run_bass_kernel_spmd

    x = np.asarray(x, dtype=np.float32)
    mask = np.asarray(mask, dtype=np.float32)
    Wq = np.asarray(Wq, dtype=np.float32)
    Wk = np.asarray(Wk, dtype=np.float32)
    Wv = np.asarray(Wv, dtype=np.float32)
    Wfc = np.asarray(Wfc, dtype=np.float32)
    bfc = np.asarray(bfc, dtype=np.float32)

    B = x.shape[0]
    with_mask = bool(np.any(mask))
    nc = get_nc(with_mask)
    in_maps = make_in_maps(x, mask, Wq, Wk, Wv, Wfc, with_mask)

    res = run_bass_kernel_spmd(nc, in_maps, core_ids=list(range(8)))
    parts = np.stack([np.asarray(r["y"], dtype=np.float64) for r in res.results])
    out = parts.reshape(B, 4, S, D).sum(axis=1)
    out += bfc.astype(np.float64)
    return out.astype(np.float32)

